# revision 4
# baseline (speedup 1.0000x reference)
"""Trainium2 Bass kernel for nn_DEAM_79044578116356 (dilated 9-neighbor local
attention block: conv1x1+BN+ReLU -> qkv -> 3x3 dil-2 neighborhood softmax
attention -> residual -> 1x1 fc).

Contract: kernel(**inputs) takes the FULL unsharded inputs (B=8) and returns
the FULL [8, 64, 128, 128] float32 output. Internally shards data-parallel
over batch across the 8 NeuronCores (weights replicated), one image per core.

Self-contained: builds the Bass program, folds BN/bias/scale into host-side
constant tensors, runs via concourse.bass_utils.run_bass_kernel_spmd.

Device layout (per core): partition = c + 64*(h%2), free = rp*W + w with
rp = h//2. dy shifts in {-2,0,2} preserve row parity, so every dilated
(dy,dx) shift of k/v is a pure free-dim offset into a zero-padded
[66 rp x 132 w] plane. All compute-engine ops are partition-aligned; conv
and fc are parity-split with explicit psum base partitions / zero-padded
weights.
"""
import os

import numpy as np

os.environ.setdefault("JAX_COMPILATION_CACHE_DIR", "/tmp/jax_neff_cache")

import concourse.bass as bass
import concourse.mybir as mybir
from concourse.bass_utils import run_bass_kernel_spmd
from concourse.tile import TileContext

# ---------------------------------------------------------------------------
# Workaround for this walrus build's 1-sync-wait-per-instruction limit
# ("Too many sync wait commands" from setupSyncWait for CTRL/S3_LW/...).
# Extra sem waits are hoisted onto same-engine InstNoOp instructions placed
# immediately before the owner (engines run in program order, so an earlier
# same-engine wait is equivalent).
# ---------------------------------------------------------------------------
import concourse.tile as _tile_mod
from concourse.vector_clock import ScopedClock as _ScopedClock

_MAX_WAITS = 1


def _split_inst_waits(nc, inst, out_list):
    si = inst.sync_info
    if si is None or not si.on_wait or len(si.on_wait) <= _MAX_WAITS:
        out_list.append(inst)
        return
    waits = list(si.on_wait)
    keep, extra = waits[:_MAX_WAITS], waits[_MAX_WAITS:]
    si.on_wait.clear()
    si.on_wait.extend(keep)
    for i in range(0, len(extra), _MAX_WAITS):
        chunk = extra[i:i + _MAX_WAITS]
        nop = mybir.InstNoOp(
            name=nc.get_next_instruction_name(),
            engine=inst.engine,
            ins=[],
            outs=[],
            sync_info=mybir.SyncInfo(on_wait=list(chunk), on_update=[]),
            bass_nofuse=True,
        )
        nc.register_instruction(nop, overwrite=True)
        out_list.append(nop)
    out_list.append(inst)


if not getattr(_tile_mod.TileContext, "_deam_wait_patch", False):
    _orig_lower = _tile_mod.TileContext._lower_ordered_insts

    def _patched_lower(self, ordered):
        nc = self.nc
        for _bb, insts in ordered.items():
            new_list = []
            for inst in insts:
                _split_inst_waits(nc, inst, new_list)
            insts[:] = new_list
        return _orig_lower(self, ordered)

    def _patched_drain_and_barrier(self, tick_clock, wait_clock):
        nc = self.nc
        drain_inst = nc.sync.drain()
        wait_clock.add_sem_waits(
            drain_inst.ins, _ScopedClock({None: tick_clock.global_clock})
        )
        inst = drain_inst.ins
        si = inst.sync_info
        if si is not None and si.on_wait and len(si.on_wait) > _MAX_WAITS:
            waits = list(si.on_wait)
            si.on_wait.clear()
            si.on_wait.extend(waits[:_MAX_WAITS])
            rest = waits[_MAX_WAITS:]
            while rest:
                chunk, rest = rest[:_MAX_WAITS], rest[_MAX_WAITS:]
                nop = nc.sync.nop(nofuse=True, hint="drain_wait_split")
                nsi = nop.ins.sync_info
                if nsi is None:
                    nop.ins.sync_info = mybir.SyncInfo(on_wait=list(chunk),
                                                       on_update=[])
                else:
                    nsi.on_wait.extend(chunk)
        nc.all_engine_barrier()
        assert self.sems is not None
        popped = nc._tile_sem_poison_stack.pop()
        assert popped is self._sem_poison
        nc.clear_and_free_semaphores(list(self.sems.allocated().values()))
        nc.all_engine_barrier()

    _tile_mod.TileContext._lower_ordered_insts = _patched_lower
    _tile_mod.TileContext._drain_and_barrier = _patched_drain_and_barrier
    _tile_mod.TileContext._deam_wait_patch = True

# ---------------------------------------------------------------------------
# Problem constants (hardcoded per the harness contract)
# ---------------------------------------------------------------------------
F32 = mybir.dt.float32
B = 8
C, H, W = 64, 128, 128
HW = H * W
RP = H // 2            # 64 row-pairs
KW = W + 4             # 132 (w padded by 2 each side)
KR = RP + 2            # 66  (rp padded by 1 each side)
KF = KR * KW
QF = RP * W
BN_EPS = 1e-5
OFFS = [(dy, dx) for dy in (-2, 0, 2) for dx in (-2, 0, 2)]
ACH = 8                # attention chunks (8 rp each)
CCH = 16               # conv chunks (8 image rows each)
MULT = mybir.AluOpType.mult
ADD = mybir.AluOpType.add


def _host_consts(conv1_w, conv1_b, bn_gamma, bn_beta, bn_mean, bn_var,
                 fc_w, fc_b):
    """Fold BN into the conv, 1/sqrt(C) into the q chunk; build the constant
    selection matrices for the on-device partition-reduce/broadcast matmuls."""
    inv = (bn_gamma / np.sqrt(bn_var + BN_EPS)).astype(np.float32)
    Wf = (conv1_w * inv[:, None]).astype(np.float32)          # [192, 64]
    bf = (conv1_b * inv + (bn_beta - bn_mean * inv)).astype(np.float32)
    scale = np.float32(1.0 / np.sqrt(np.float32(C)))
    WQ = np.zeros((65, 64), np.float32)
    WQ[0:64] = Wf[0:64].T * scale
    WQ[64] = bf[0:64] * scale
    WK = np.zeros((65, 64), np.float32)
    WK[0:64] = Wf[64:128].T
    WK[64] = bf[64:128]
    WV = np.zeros((65, 64), np.float32)
    WV[0:64] = Wf[128:192].T
    WV[64] = bf[128:192]
    FCE = np.zeros((128, 64), np.float32)
    FCO = np.zeros((128, 64), np.float32)
    FCE[0:64] = fc_w.T.astype(np.float32)
    FCO[64:128] = fc_w.T.astype(np.float32)
    FCB = fc_b.reshape(64, 1).astype(np.float32)
    ones9 = np.zeros((128, 9 * 18), np.float32)
    for n in range(9):
        for par in range(2):
            ones9[64 * par:64 * par + 64, 18 * n + 2 * n + par] = 1.0
    zsel = np.zeros((18, 2), np.float32)
    for p in range(18):
        zsel[p, p % 2] = 1.0
    zrep = np.zeros((2, 18), np.float32)
    for p in range(18):
        zrep[p % 2, p] = 1.0
    bsel = np.zeros((18, 9 * 128), np.float32)
    for n in range(9):
        for j in range(128):
            bsel[2 * n + j // 64, 128 * n + j] = 1.0
    return dict(WQ=WQ, WK=WK, WV=WV, FCE=FCE, FCO=FCO, FCB=FCB, ONES9=ones9,
                ZSEL=zsel, ZREP=zrep, BSEL=bsel)


def build(nc: bass.Bass, qk_dt=mybir.dt.float32):
    ei = lambda n, s: nc.dram_tensor(n, s, F32, kind="ExternalInput")
    e_map = ei("e_map", [C, H, W])
    f_map = ei("f_map", [C, H, W])
    WQ, WK, WV = ei("WQ", [65, 64]), ei("WK", [65, 64]), ei("WV", [65, 64])
    FCE, FCO, FCB = ei("FCE", [128, 64]), ei("FCO", [128, 64]), ei("FCB", [64, 1])
    ONES9 = nc.dram_tensor("ONES9", [128, 9 * 18], qk_dt, kind="ExternalInput")
    ZSEL, ZREP = ei("ZSEL", [18, 2]), ei("ZREP", [2, 18])
    BSEL = ei("BSEL", [18, 9 * 128])
    y = nc.dram_tensor("y", [C, H, W], F32, kind="ExternalOutput")

    with TileContext(nc) as tc:
        with tc.tile_pool(name="persist", bufs=1) as P:
            q2 = P.tile([128, QF], qk_dt, tag="q2")
            k2 = P.tile([128, KF], qk_dt, tag="k2")
            v2 = P.tile([128, KF], F32, tag="v2")
            wq = P.tile([65, 64], F32, tag="wq")
            wk = P.tile([65, 64], F32, tag="wk")
            wv = P.tile([65, 64], F32, tag="wv")
            fce = P.tile([128, 64], F32, tag="fce")
            fco = P.tile([128, 64], F32, tag="fco")
            fcb = P.tile([64, 1], F32, tag="fcb")
            ones_t = P.tile([128, 9 * 18], qk_dt, tag="ones")
            zsel_t = P.tile([18, 2], F32, tag="zsel")
            zrep_t = P.tile([2, 18], F32, tag="zrep")
            bsel_t = P.tile([18, 9 * 128], F32, tag="bsel")

            for t, d in ((wq, WQ), (wk, WK), (wv, WV), (fce, FCE), (fco, FCO),
                         (fcb, FCB), (ones_t, ONES9), (zsel_t, ZSEL),
                         (zrep_t, ZREP), (bsel_t, BSEL)):
                nc.sync.dma_start(t[:, :], d[:, :])
            nc.gpsimd.memset(k2[:, :], 0.0)
            nc.gpsimd.memset(v2[:, :], 0.0)

            q2r = q2[:, :].rearrange("p (r w) -> p r w", w=W)
            k2r = k2[:, :].rearrange("p (r w) -> p r w", w=KW)
            v2r = v2[:, :].rearrange("p (r w) -> p r w", w=KW)

            # ---------------- conv phase (parity-split) ----------------
            with tc.tile_pool(name="est", bufs=1) as E:
                est = E.tile([65, HW], F32, tag="est")
                nc.sync.dma_start(est[0:64, :], e_map[:, :, :])
                nc.gpsimd.memset(est[64:65, :], 1.0)
                estr = est[:, :].rearrange("p (h w) -> p h w", w=W)
                with tc.tile_pool(name="cps", bufs=2, space="PSUM") as CP:
                    nrow = H // CCH              # 8 rows per chunk
                    nr2 = nrow // 2              # 4 row-pairs
                    cfa = nr2 * W                # 512 packed cols
                    for ch in range(CCH):
                        h0 = ch * nrow
                        rp0 = h0 // 2
                        pq = CP.tile([128, cfa], F32, tag="pq")
                        pk = CP.tile([128, cfa], F32, tag="pk")
                        pv = CP.tile([128, cfa], F32, tag="pv")
                        for par in (0, 1):
                            rhs = estr[:, h0 + par:h0 + nrow:2, :]
                            ps = slice(64 * par, 64 * par + 64)
                            nc.tensor.matmul(pq[ps, :], wq[:, :], rhs,
                                             start=True, stop=True)
                            nc.tensor.matmul(pk[ps, :], wk[:, :], rhs,
                                             start=True, stop=True)
                            nc.tensor.matmul(pv[ps, :], wv[:, :], rhs,
                                             start=True, stop=True)
                        nc.vector.tensor_scalar_max(
                            q2r[:, rp0:rp0 + nr2, :],
                            pq[:, :].rearrange("p (r w) -> p r w", w=W), 0.0)
                        nc.vector.tensor_scalar_max(
                            k2r[:, rp0 + 1:rp0 + 1 + nr2, 2:2 + W],
                            pk[:, :].rearrange("p (r w) -> p r w", w=W), 0.0)
                        nc.vector.tensor_scalar_max(
                            v2r[:, rp0 + 1:rp0 + 1 + nr2, 2:2 + W],
                            pv[:, :].rearrange("p (r w) -> p r w", w=W), 0.0)

            # ---------------- attention + residual + fc ----------------
            with tc.tile_pool(name="xp", bufs=1) as X:
                xp = X.tile([128, QF], F32, tag="xp")
                for par in (0, 1):
                    nc.sync.dma_start(
                        xp[64 * par:64 * par + 64, :].rearrange(
                            "p (r w) -> p r w", w=W),
                        f_map[:, par:H:2, :])

                crp = RP // ACH            # 8 row-pairs per chunk
                fa = crp * W               # 1024 packed cols
                with tc.tile_pool(name="aps", bufs=1, space="PSUM") as APS, \
                     tc.tile_pool(name="bps", bufs=2, space="PSUM") as BPS, \
                     tc.tile_pool(name="fps", bufs=1, space="PSUM") as FPS, \
                     tc.tile_pool(name="asb", bufs=2) as ASB, \
                     tc.tile_pool(name="psb", bufs=2) as PSB, \
                     tc.tile_pool(name="osb", bufs=2) as OSB:
                    for ch in range(ACH):
                        rp0 = ch * crp
                        # scores: 9 shifted q*k products -> ones-matmul
                        # partition-reduce, psum-accumulated into [18, fa]
                        s_ps = APS.tile([18, fa], F32, tag="A")
                        for n, (dy, dx) in enumerate(OFFS):
                            s = dy // 2
                            prod = PSB.tile([128, fa], qk_dt, tag="prod")
                            nc.vector.tensor_tensor(
                                prod[:, :].rearrange("p (r w) -> p r w", w=W),
                                q2r[:, rp0:rp0 + crp, :],
                                k2r[:, rp0 + 1 + s:rp0 + 1 + s + crp,
                                    2 + dx:2 + dx + W],
                                MULT)
                            for b in range(fa // 512):
                                nc.tensor.matmul(
                                    s_ps[:, b * 512:(b + 1) * 512],
                                    ones_t[:, 18 * n:18 * n + 18],
                                    prod[:, b * 512:(b + 1) * 512],
                                    start=(n == 0), stop=(n == 8))
                        # softmax over the 9 neighbors (scores are bounded
                        # small; exp without max-subtraction is safe in f32)
                        e_sb = ASB.tile([18, fa], F32, tag="e")
                        nc.scalar.activation(e_sb[:, :], s_ps[:, :],
                                             mybir.ActivationFunctionType.Exp)
                        z_ps = BPS.tile([2, fa], F32, tag="B")
                        for b in range(fa // 512):
                            nc.tensor.matmul(z_ps[:, b * 512:(b + 1) * 512],
                                             zsel_t[:, :],
                                             e_sb[:, b * 512:(b + 1) * 512],
                                             start=True, stop=True)
                        zr_sb = ASB.tile([2, fa], F32, tag="zr")
                        nc.vector.reciprocal(zr_sb[:, :], z_ps[:, :])
                        zrep_ps = APS.tile([18, fa], F32, tag="A")
                        for b in range(fa // 512):
                            nc.tensor.matmul(zrep_ps[:, b * 512:(b + 1) * 512],
                                             zrep_t[:, :],
                                             zr_sb[:, b * 512:(b + 1) * 512],
                                             start=True, stop=True)
                        a_sb = ASB.tile([18, fa], F32, tag="a")
                        nc.vector.tensor_tensor(a_sb[:, :], e_sb[:, :],
                                                zrep_ps[:, :], MULT)
                        # AV: per-n broadcast matmul + product + accumulate
                        acc = PSB.tile([128, fa], F32, tag="acc")
                        for n, (dy, dx) in enumerate(OFFS):
                            s = dy // 2
                            abc = BPS.tile([128, fa], F32, tag="B")
                            for b in range(fa // 512):
                                nc.tensor.matmul(
                                    abc[:, b * 512:(b + 1) * 512],
                                    bsel_t[:, 128 * n:128 * n + 128],
                                    a_sb[:, b * 512:(b + 1) * 512],
                                    start=True, stop=True)
                            vsh = v2r[:, rp0 + 1 + s:rp0 + 1 + s + crp,
                                      2 + dx:2 + dx + W]
                            dst = acc if n == 0 else PSB.tile([128, fa], F32,
                                                              tag="prod")
                            nc.vector.tensor_tensor(
                                dst[:, :].rearrange("p (r w) -> p r w", w=W),
                                abc[:, :].rearrange("p (r w) -> p r w", w=W),
                                vsh, MULT)
                            if n > 0:
                                nc.vector.tensor_tensor(
                                    acc[:, :], acc[:, :], dst[:, :], ADD)
                        # residual (aligned packed add)
                        nc.vector.tensor_tensor(
                            xp[:, rp0 * W:(rp0 + crp) * W],
                            xp[:, rp0 * W:(rp0 + crp) * W], acc[:, :], ADD)
                        # fc (parity-split), bias at evac, strided DMA out
                        for par, fcw in ((0, fce), (1, fco)):
                            fc_ps = FPS.tile([64, fa], F32, tag="fc")
                            for b in range(fa // 512):
                                nc.tensor.matmul(
                                    fc_ps[:, b * 512:(b + 1) * 512], fcw[:, :],
                                    xp[:, rp0 * W + b * 512:
                                       rp0 * W + (b + 1) * 512],
                                    start=True, stop=True)
                            ob = OSB.tile([64, fa], F32, tag="ob")
                            nc.vector.tensor_scalar_add(ob[:, :], fc_ps[:, :],
                                                        fcb[:, 0:1])
                            nc.sync.dma_start(
                                y[:, 2 * rp0 + par:2 * (rp0 + crp):2, :],
                                ob[:, :].rearrange("p (r w) -> p r w", w=W))
    return nc


_build_cache = {}


def _get_nc():
    if "nc" not in _build_cache:
        nc = bass.Bass()
        build(nc)
        _build_cache["nc"] = nc
    return _build_cache["nc"]


def run_spmd(in_maps, **kw):
    """Run the prebuilt program on cores 0..len(in_maps)-1."""
    nc = _get_nc()
    return run_bass_kernel_spmd(nc, in_maps, core_ids=list(range(len(in_maps))),
                                **kw)


def make_in_maps(f_map, e_map, conv1_w, conv1_b, bn_gamma, bn_beta, bn_mean,
                 bn_var, fc_w, fc_b):
    consts = _host_consts(np.asarray(conv1_w), np.asarray(conv1_b),
                          np.asarray(bn_gamma), np.asarray(bn_beta),
                          np.asarray(bn_mean), np.asarray(bn_var),
                          np.asarray(fc_w), np.asarray(fc_b))
    f_map = np.ascontiguousarray(np.asarray(f_map, dtype=np.float32))
    e_map = np.ascontiguousarray(np.asarray(e_map, dtype=np.float32))
    return [dict(e_map=e_map[b], f_map=f_map[b], **consts) for b in range(B)]


def kernel(f_map, e_map, conv1_w, conv1_b, bn_gamma, bn_beta, bn_mean, bn_var,
           fc_w, fc_b):
    in_maps = make_in_maps(f_map, e_map, conv1_w, conv1_b, bn_gamma, bn_beta,
                           bn_mean, bn_var, fc_w, fc_b)
    res = run_spmd(in_maps)
    out = np.stack([res.results[b]["y"] for b in range(B)]).astype(np.float32)
    return out


# revision 5
# speedup vs baseline: 5434.1531x; 5434.1531x over previous
"""Trainium2 Bass kernel for nn_DEAM_79044578116356 (dilated 9-neighbor local
attention block: conv1x1+BN+ReLU -> qkv -> 3x3 dil-2 neighborhood softmax
attention -> residual -> 1x1 fc).

Contract: kernel(**inputs) takes the FULL unsharded inputs (B=8) and returns
the FULL [8, 64, 128, 128] float32 output. Internally shards data-parallel
over batch across the 8 NeuronCores (weights replicated), one image per core.

Self-contained: builds the Bass program, folds BN/bias/scale into host-side
constant tensors, runs via concourse.bass_utils.run_bass_kernel_spmd.

Device layout (per core): partition = c + 64*(h%2), free = rp*W + w with
rp = h//2. dy shifts in {-2,0,2} preserve row parity, so every dilated
(dy,dx) shift of k/v is a pure free-dim offset into a zero-padded
[66 rp x 132 w] plane. All compute-engine ops are partition-aligned; conv
and fc are parity-split with explicit psum base partitions / zero-padded
weights.
"""
import os

import numpy as np

os.environ.setdefault("JAX_COMPILATION_CACHE_DIR", "/tmp/jax_neff_cache")

import concourse.bass as bass
import concourse.mybir as mybir
from concourse.bass_utils import run_bass_kernel_spmd
from concourse.tile import TileContext

# ---------------------------------------------------------------------------
# Workaround for this walrus build's 1-sync-wait-per-instruction limit
# ("Too many sync wait commands" from setupSyncWait for CTRL/S3_LW/...).
# Extra sem waits are hoisted onto same-engine InstNoOp instructions placed
# immediately before the owner (engines run in program order, so an earlier
# same-engine wait is equivalent).
# ---------------------------------------------------------------------------
import concourse.tile as _tile_mod
from concourse.vector_clock import ScopedClock as _ScopedClock

_MAX_WAITS = 1


def _split_inst_waits(nc, inst, out_list):
    si = inst.sync_info
    if si is None or not si.on_wait or len(si.on_wait) <= _MAX_WAITS:
        out_list.append(inst)
        return
    waits = list(si.on_wait)
    keep, extra = waits[:_MAX_WAITS], waits[_MAX_WAITS:]
    si.on_wait.clear()
    si.on_wait.extend(keep)
    for i in range(0, len(extra), _MAX_WAITS):
        chunk = extra[i:i + _MAX_WAITS]
        nop = mybir.InstNoOp(
            name=nc.get_next_instruction_name(),
            engine=inst.engine,
            ins=[],
            outs=[],
            sync_info=mybir.SyncInfo(on_wait=list(chunk), on_update=[]),
            bass_nofuse=True,
        )
        nc.register_instruction(nop, overwrite=True)
        out_list.append(nop)
    out_list.append(inst)


if not getattr(_tile_mod.TileContext, "_deam_wait_patch", False):
    _orig_lower = _tile_mod.TileContext._lower_ordered_insts

    def _patched_lower(self, ordered):
        nc = self.nc
        for _bb, insts in ordered.items():
            new_list = []
            for inst in insts:
                _split_inst_waits(nc, inst, new_list)
            insts[:] = new_list
        return _orig_lower(self, ordered)

    def _patched_drain_and_barrier(self, tick_clock, wait_clock):
        nc = self.nc
        drain_inst = nc.sync.drain()
        wait_clock.add_sem_waits(
            drain_inst.ins, _ScopedClock({None: tick_clock.global_clock})
        )
        inst = drain_inst.ins
        si = inst.sync_info
        if si is not None and si.on_wait and len(si.on_wait) > _MAX_WAITS:
            waits = list(si.on_wait)
            si.on_wait.clear()
            si.on_wait.extend(waits[:_MAX_WAITS])
            rest = waits[_MAX_WAITS:]
            while rest:
                chunk, rest = rest[:_MAX_WAITS], rest[_MAX_WAITS:]
                nop = nc.sync.nop(nofuse=True, hint="drain_wait_split")
                nsi = nop.ins.sync_info
                if nsi is None:
                    nop.ins.sync_info = mybir.SyncInfo(on_wait=list(chunk),
                                                       on_update=[])
                else:
                    nsi.on_wait.extend(chunk)
        nc.all_engine_barrier()
        assert self.sems is not None
        popped = nc._tile_sem_poison_stack.pop()
        assert popped is self._sem_poison
        nc.clear_and_free_semaphores(list(self.sems.allocated().values()))
        nc.all_engine_barrier()

    _tile_mod.TileContext._lower_ordered_insts = _patched_lower
    _tile_mod.TileContext._drain_and_barrier = _patched_drain_and_barrier
    _tile_mod.TileContext._deam_wait_patch = True

# ---------------------------------------------------------------------------
# Problem constants (hardcoded per the harness contract)
# ---------------------------------------------------------------------------
F32 = mybir.dt.float32
B = 8
C, H, W = 64, 128, 128
HW = H * W
RP = H // 2            # 64 row-pairs
KW = W + 4             # 132 (w padded by 2 each side)
KR = RP + 2            # 66  (rp padded by 1 each side)
KF = KR * KW
QF = RP * W
BN_EPS = 1e-5
OFFS = [(dy, dx) for dy in (-2, 0, 2) for dx in (-2, 0, 2)]
ACH = 16               # attention chunks (4 rp each)
CCH = 16               # conv chunks (8 image rows each)
MULT = mybir.AluOpType.mult
ADD = mybir.AluOpType.add


def _host_consts(conv1_w, conv1_b, bn_gamma, bn_beta, bn_mean, bn_var,
                 fc_w, fc_b):
    """Fold BN into the conv, 1/sqrt(C) into the q chunk; build the constant
    selection matrices for the on-device partition-reduce/broadcast matmuls."""
    inv = (bn_gamma / np.sqrt(bn_var + BN_EPS)).astype(np.float32)
    Wf = (conv1_w * inv[:, None]).astype(np.float32)          # [192, 64]
    bf = (conv1_b * inv + (bn_beta - bn_mean * inv)).astype(np.float32)
    scale = np.float32(1.0 / np.sqrt(np.float32(C)))
    WQ = np.zeros((65, 64), np.float32)
    WQ[0:64] = Wf[0:64].T * scale
    WQ[64] = bf[0:64] * scale
    WK = np.zeros((65, 64), np.float32)
    WK[0:64] = Wf[64:128].T
    WK[64] = bf[64:128]
    WV = np.zeros((65, 64), np.float32)
    WV[0:64] = Wf[128:192].T
    WV[64] = bf[128:192]
    FCE = np.zeros((128, 64), np.float32)
    FCO = np.zeros((128, 64), np.float32)
    FCE[0:64] = fc_w.T.astype(np.float32)
    FCO[64:128] = fc_w.T.astype(np.float32)
    FCB = fc_b.reshape(64, 1).astype(np.float32)
    ones9 = np.zeros((128, 9 * 18), np.float32)
    for n in range(9):
        for par in range(2):
            ones9[64 * par:64 * par + 64, 18 * n + 2 * n + par] = 1.0
    zsel = np.zeros((18, 2), np.float32)
    for p in range(18):
        zsel[p, p % 2] = 1.0
    zrep = np.zeros((2, 18), np.float32)
    for p in range(18):
        zrep[p % 2, p] = 1.0
    bsel = np.zeros((18, 9 * 128), np.float32)
    for n in range(9):
        for j in range(128):
            bsel[2 * n + j // 64, 128 * n + j] = 1.0
    return dict(WQ=WQ, WK=WK, WV=WV, FCE=FCE, FCO=FCO, FCB=FCB, ONES9=ones9,
                ZSEL=zsel, ZREP=zrep, BSEL=bsel)


def build(nc: bass.Bass, qk_dt=mybir.dt.float32):
    ei = lambda n, s: nc.dram_tensor(n, s, F32, kind="ExternalInput")
    e_map = ei("e_map", [C, H, W])
    f_map = ei("f_map", [C, H, W])
    WQ, WK, WV = ei("WQ", [65, 64]), ei("WK", [65, 64]), ei("WV", [65, 64])
    FCE, FCO, FCB = ei("FCE", [128, 64]), ei("FCO", [128, 64]), ei("FCB", [64, 1])
    ONES9 = nc.dram_tensor("ONES9", [128, 9 * 18], qk_dt, kind="ExternalInput")
    ZSEL, ZREP = ei("ZSEL", [18, 2]), ei("ZREP", [2, 18])
    BSEL = ei("BSEL", [18, 9 * 128])
    y = nc.dram_tensor("y", [C, H, W], F32, kind="ExternalOutput")

    with TileContext(nc) as tc:
        with tc.tile_pool(name="persist", bufs=1) as P:
            q2 = P.tile([128, QF], qk_dt, tag="q2")
            k2 = P.tile([128, KF], qk_dt, tag="k2")
            v2 = P.tile([128, KF], F32, tag="v2")
            wq = P.tile([65, 64], F32, tag="wq")
            wk = P.tile([65, 64], F32, tag="wk")
            wv = P.tile([65, 64], F32, tag="wv")
            fce = P.tile([128, 64], F32, tag="fce")
            fco = P.tile([128, 64], F32, tag="fco")
            fcb = P.tile([64, 1], F32, tag="fcb")
            ones_t = P.tile([128, 9 * 18], qk_dt, tag="ones")
            zsel_t = P.tile([18, 2], F32, tag="zsel")
            zrep_t = P.tile([2, 18], F32, tag="zrep")
            bsel_t = P.tile([18, 9 * 128], F32, tag="bsel")

            for t, d in ((wq, WQ), (wk, WK), (wv, WV), (fce, FCE), (fco, FCO),
                         (fcb, FCB), (ones_t, ONES9), (zsel_t, ZSEL),
                         (zrep_t, ZREP), (bsel_t, BSEL)):
                nc.sync.dma_start(t[:, :], d[:, :])
            nc.gpsimd.memset(k2[:, :], 0.0)
            nc.gpsimd.memset(v2[:, :], 0.0)

            q2r = q2[:, :].rearrange("p (r w) -> p r w", w=W)
            k2r = k2[:, :].rearrange("p (r w) -> p r w", w=KW)
            v2r = v2[:, :].rearrange("p (r w) -> p r w", w=KW)

            # ---------------- conv phase (parity-split) ----------------
            with tc.tile_pool(name="est", bufs=1) as E:
                est = E.tile([65, HW], F32, tag="est")
                nc.sync.dma_start(est[0:64, :], e_map[:, :, :])
                nc.gpsimd.memset(est[64:65, :], 1.0)
                estr = est[:, :].rearrange("p (h w) -> p h w", w=W)
                with tc.tile_pool(name="cps", bufs=2, space="PSUM") as CP:
                    nrow = H // CCH              # 8 rows per chunk
                    nr2 = nrow // 2              # 4 row-pairs
                    cfa = nr2 * W                # 512 packed cols
                    for ch in range(CCH):
                        h0 = ch * nrow
                        rp0 = h0 // 2
                        pq = CP.tile([128, cfa], F32, tag="pq")
                        pk = CP.tile([128, cfa], F32, tag="pk")
                        pv = CP.tile([128, cfa], F32, tag="pv")
                        for par in (0, 1):
                            rhs = estr[:, h0 + par:h0 + nrow:2, :]
                            ps = slice(64 * par, 64 * par + 64)
                            nc.tensor.matmul(pq[ps, :], wq[:, :], rhs,
                                             start=True, stop=True)
                            nc.tensor.matmul(pk[ps, :], wk[:, :], rhs,
                                             start=True, stop=True)
                            nc.tensor.matmul(pv[ps, :], wv[:, :], rhs,
                                             start=True, stop=True)
                        nc.scalar.activation(
                            q2r[:, rp0:rp0 + nr2, :],
                            pq[:, :].rearrange("p (r w) -> p r w", w=W),
                            mybir.ActivationFunctionType.Relu)
                        nc.scalar.activation(
                            k2r[:, rp0 + 1:rp0 + 1 + nr2, 2:2 + W],
                            pk[:, :].rearrange("p (r w) -> p r w", w=W),
                            mybir.ActivationFunctionType.Relu)
                        nc.vector.tensor_scalar_max(
                            v2r[:, rp0 + 1:rp0 + 1 + nr2, 2:2 + W],
                            pv[:, :].rearrange("p (r w) -> p r w", w=W), 0.0)

            # ---------------- attention + residual + fc ----------------
            with tc.tile_pool(name="xp", bufs=1) as X:
                xp = X.tile([128, QF], F32, tag="xp")
                for par in (0, 1):
                    nc.sync.dma_start(
                        xp[64 * par:64 * par + 64, :].rearrange(
                            "p (r w) -> p r w", w=W),
                        f_map[:, par:H:2, :])

                crp = RP // ACH            # 8 row-pairs per chunk
                fa = crp * W               # 1024 packed cols
                with tc.tile_pool(name="aps", bufs=2, space="PSUM") as APS, \
                     tc.tile_pool(name="bps", bufs=3, space="PSUM") as BPS, \
                     tc.tile_pool(name="fps", bufs=2, space="PSUM") as FPS, \
                     tc.tile_pool(name="asb", bufs=3) as ASB, \
                     tc.tile_pool(name="psb", bufs=4) as PSB, \
                     tc.tile_pool(name="osb", bufs=2) as OSB:
                    for ch in range(ACH):
                        rp0 = ch * crp
                        # scores: 9 shifted q*k products -> ones-matmul
                        # partition-reduce, psum-accumulated into [18, fa]
                        s_ps = APS.tile([18, fa], F32, tag="A")
                        for n, (dy, dx) in enumerate(OFFS):
                            s = dy // 2
                            prod = PSB.tile([128, fa], qk_dt, tag="prod")
                            nc.vector.tensor_tensor(
                                prod[:, :].rearrange("p (r w) -> p r w", w=W),
                                q2r[:, rp0:rp0 + crp, :],
                                k2r[:, rp0 + 1 + s:rp0 + 1 + s + crp,
                                    2 + dx:2 + dx + W],
                                MULT)
                            for b in range(fa // 512):
                                nc.tensor.matmul(
                                    s_ps[:, b * 512:(b + 1) * 512],
                                    ones_t[:, 18 * n:18 * n + 18],
                                    prod[:, b * 512:(b + 1) * 512],
                                    start=(n == 0), stop=(n == 8))
                        # softmax over the 9 neighbors (scores are bounded
                        # small; exp without max-subtraction is safe in f32)
                        e_sb = ASB.tile([18, fa], F32, tag="e")
                        nc.scalar.activation(e_sb[:, :], s_ps[:, :],
                                             mybir.ActivationFunctionType.Exp)
                        z_ps = BPS.tile([2, fa], F32, tag="B")
                        for b in range(fa // 512):
                            nc.tensor.matmul(z_ps[:, b * 512:(b + 1) * 512],
                                             zsel_t[:, :],
                                             e_sb[:, b * 512:(b + 1) * 512],
                                             start=True, stop=True)
                        zr_sb = ASB.tile([2, fa], F32, tag="zr")
                        nc.vector.reciprocal(zr_sb[:, :], z_ps[:, :])
                        zrep_ps = APS.tile([18, fa], F32, tag="A")
                        for b in range(fa // 512):
                            nc.tensor.matmul(zrep_ps[:, b * 512:(b + 1) * 512],
                                             zrep_t[:, :],
                                             zr_sb[:, b * 512:(b + 1) * 512],
                                             start=True, stop=True)
                        a_sb = ASB.tile([18, fa], F32, tag="a")
                        nc.vector.tensor_tensor(a_sb[:, :], e_sb[:, :],
                                                zrep_ps[:, :], MULT)
                        # AV: per-n broadcast matmul + product + accumulate
                        acc = PSB.tile([128, fa], F32, tag="acc")
                        accB = PSB.tile([128, fa], F32, tag="accB")
                        for n, (dy, dx) in enumerate(OFFS):
                            s = dy // 2
                            abc = BPS.tile([128, fa], F32, tag="B")
                            for b in range(fa // 512):
                                nc.tensor.matmul(
                                    abc[:, b * 512:(b + 1) * 512],
                                    bsel_t[:, 128 * n:128 * n + 128],
                                    a_sb[:, b * 512:(b + 1) * 512],
                                    start=True, stop=True)
                            vsh = v2r[:, rp0 + 1 + s:rp0 + 1 + s + crp,
                                      2 + dx:2 + dx + W]
                            dst = (acc if n == 0 else
                                   accB if n == 1 else
                                   PSB.tile([128, fa], F32, tag="prod"))
                            nc.vector.tensor_tensor(
                                dst[:, :].rearrange("p (r w) -> p r w", w=W),
                                abc[:, :].rearrange("p (r w) -> p r w", w=W),
                                vsh, MULT)
                            if n >= 2:
                                if n % 2 == 0:
                                    nc.vector.tensor_tensor(
                                        acc[:, :], acc[:, :], dst[:, :], ADD)
                                else:
                                    nc.gpsimd.tensor_tensor(
                                        accB[:, :], accB[:, :], dst[:, :], ADD)
                        nc.gpsimd.tensor_tensor(acc[:, :], acc[:, :],
                                                accB[:, :], ADD)
                        # residual (aligned packed add)
                        nc.gpsimd.tensor_tensor(
                            xp[:, rp0 * W:(rp0 + crp) * W],
                            xp[:, rp0 * W:(rp0 + crp) * W], acc[:, :], ADD)
                        # fc (parity-split), bias at evac, strided DMA out
                        for par, fcw in ((0, fce), (1, fco)):
                            fc_ps = FPS.tile([64, fa], F32, tag="fc")
                            for b in range(fa // 512):
                                nc.tensor.matmul(
                                    fc_ps[:, b * 512:(b + 1) * 512], fcw[:, :],
                                    xp[:, rp0 * W + b * 512:
                                       rp0 * W + (b + 1) * 512],
                                    start=True, stop=True)
                            ob = OSB.tile([64, fa], F32, tag="ob")
                            nc.scalar.activation(
                                ob[:, :], fc_ps[:, :],
                                mybir.ActivationFunctionType.Identity,
                                bias=fcb[:, 0:1])
                            nc.sync.dma_start(
                                y[:, 2 * rp0 + par:2 * (rp0 + crp):2, :],
                                ob[:, :].rearrange("p (r w) -> p r w", w=W))
    return nc


_build_cache = {}


def _get_nc():
    if "nc" not in _build_cache:
        nc = bass.Bass()
        build(nc)
        _build_cache["nc"] = nc
    return _build_cache["nc"]


def run_spmd(in_maps, **kw):
    """Run the prebuilt program on cores 0..len(in_maps)-1."""
    nc = _get_nc()
    return run_bass_kernel_spmd(nc, in_maps, core_ids=list(range(len(in_maps))),
                                **kw)


def make_in_maps(f_map, e_map, conv1_w, conv1_b, bn_gamma, bn_beta, bn_mean,
                 bn_var, fc_w, fc_b):
    consts = _host_consts(np.asarray(conv1_w), np.asarray(conv1_b),
                          np.asarray(bn_gamma), np.asarray(bn_beta),
                          np.asarray(bn_mean), np.asarray(bn_var),
                          np.asarray(fc_w), np.asarray(fc_b))
    f_map = np.ascontiguousarray(np.asarray(f_map, dtype=np.float32))
    e_map = np.ascontiguousarray(np.asarray(e_map, dtype=np.float32))
    return [dict(e_map=e_map[b], f_map=f_map[b], **consts) for b in range(B)]


def kernel(f_map, e_map, conv1_w, conv1_b, bn_gamma, bn_beta, bn_mean, bn_var,
           fc_w, fc_b):
    in_maps = make_in_maps(f_map, e_map, conv1_w, conv1_b, bn_gamma, bn_beta,
                           bn_mean, bn_var, fc_w, fc_b)
    res = run_spmd(in_maps)
    out = np.stack([res.results[b]["y"] for b in range(B)]).astype(np.float32)
    return out


# revision 6
# speedup vs baseline: 6424.3306x; 1.1822x over previous
"""Trainium2 Bass kernel for nn_DEAM_79044578116356 (dilated 9-neighbor local
attention block: conv1x1+BN+ReLU -> qkv -> 3x3 dil-2 neighborhood softmax
attention -> residual -> 1x1 fc).

Contract: kernel(**inputs) takes the FULL unsharded inputs (B=8) and returns
the FULL [8, 64, 128, 128] float32 output. Internally shards data-parallel
over batch across the 8 NeuronCores (weights replicated), one image per core.

Self-contained: builds the Bass program, folds BN/bias/scale into host-side
constant tensors, runs via concourse.bass_utils.run_bass_kernel_spmd.

Device layout (per core): partition = c + 64*(h%2), free = rp*W + w with
rp = h//2. dy shifts in {-2,0,2} preserve row parity, so every dilated
(dy,dx) shift of k/v is a pure free-dim offset into a zero-padded
[66 rp x 132 w] plane. All compute-engine ops are partition-aligned; conv
and fc are parity-split with explicit psum base partitions / zero-padded
weights.
"""
import os

import numpy as np

os.environ.setdefault("JAX_COMPILATION_CACHE_DIR", "/tmp/jax_neff_cache")

import concourse.bass as bass
import concourse.mybir as mybir
from concourse.bass_utils import run_bass_kernel_spmd
from concourse.tile import TileContext

# ---------------------------------------------------------------------------
# Workaround for this walrus build's 1-sync-wait-per-instruction limit
# ("Too many sync wait commands" from setupSyncWait for CTRL/S3_LW/...).
# Extra sem waits are hoisted onto same-engine InstNoOp instructions placed
# immediately before the owner (engines run in program order, so an earlier
# same-engine wait is equivalent).
# ---------------------------------------------------------------------------
import concourse.tile as _tile_mod
from concourse.vector_clock import ScopedClock as _ScopedClock

_MAX_WAITS = 1


def _split_inst_waits(nc, inst, out_list):
    si = inst.sync_info
    if si is None or not si.on_wait or len(si.on_wait) <= _MAX_WAITS:
        out_list.append(inst)
        return
    waits = list(si.on_wait)
    keep, extra = waits[:_MAX_WAITS], waits[_MAX_WAITS:]
    si.on_wait.clear()
    si.on_wait.extend(keep)
    for i in range(0, len(extra), _MAX_WAITS):
        chunk = extra[i:i + _MAX_WAITS]
        nop = mybir.InstNoOp(
            name=nc.get_next_instruction_name(),
            engine=inst.engine,
            ins=[],
            outs=[],
            sync_info=mybir.SyncInfo(on_wait=list(chunk), on_update=[]),
            bass_nofuse=True,
        )
        nc.register_instruction(nop, overwrite=True)
        out_list.append(nop)
    out_list.append(inst)


if not getattr(_tile_mod.TileContext, "_deam_wait_patch", False):
    _orig_lower = _tile_mod.TileContext._lower_ordered_insts

    def _patched_lower(self, ordered):
        nc = self.nc
        for _bb, insts in ordered.items():
            new_list = []
            for inst in insts:
                _split_inst_waits(nc, inst, new_list)
            insts[:] = new_list
        return _orig_lower(self, ordered)

    def _patched_drain_and_barrier(self, tick_clock, wait_clock):
        nc = self.nc
        drain_inst = nc.sync.drain()
        wait_clock.add_sem_waits(
            drain_inst.ins, _ScopedClock({None: tick_clock.global_clock})
        )
        inst = drain_inst.ins
        si = inst.sync_info
        if si is not None and si.on_wait and len(si.on_wait) > _MAX_WAITS:
            waits = list(si.on_wait)
            si.on_wait.clear()
            si.on_wait.extend(waits[:_MAX_WAITS])
            rest = waits[_MAX_WAITS:]
            while rest:
                chunk, rest = rest[:_MAX_WAITS], rest[_MAX_WAITS:]
                nop = nc.sync.nop(nofuse=True, hint="drain_wait_split")
                nsi = nop.ins.sync_info
                if nsi is None:
                    nop.ins.sync_info = mybir.SyncInfo(on_wait=list(chunk),
                                                       on_update=[])
                else:
                    nsi.on_wait.extend(chunk)
        nc.all_engine_barrier()
        assert self.sems is not None
        popped = nc._tile_sem_poison_stack.pop()
        assert popped is self._sem_poison
        nc.clear_and_free_semaphores(list(self.sems.allocated().values()))
        nc.all_engine_barrier()

    _tile_mod.TileContext._lower_ordered_insts = _patched_lower
    _tile_mod.TileContext._drain_and_barrier = _patched_drain_and_barrier
    _tile_mod.TileContext._deam_wait_patch = True

# ---------------------------------------------------------------------------
# Problem constants (hardcoded per the harness contract)
# ---------------------------------------------------------------------------
F32 = mybir.dt.float32
B = 8
C, H, W = 64, 128, 128
HW = H * W
RP = H // 2            # 64 row-pairs
KW = W + 4             # 132 (w padded by 2 each side)
KR = RP + 2            # 66  (rp padded by 1 each side)
KF = KR * KW
QF = RP * W
BN_EPS = 1e-5
OFFS = [(dy, dx) for dy in (-2, 0, 2) for dx in (-2, 0, 2)]
ACH = 16               # attention chunks (4 rp each)
CCH = 16               # conv chunks (8 image rows each)
MULT = mybir.AluOpType.mult
ADD = mybir.AluOpType.add


def _host_consts(conv1_w, conv1_b, bn_gamma, bn_beta, bn_mean, bn_var,
                 fc_w, fc_b):
    """Fold BN into the conv, 1/sqrt(C) into the q chunk; build the constant
    selection matrices for the on-device partition-reduce/broadcast matmuls."""
    inv = (bn_gamma / np.sqrt(bn_var + BN_EPS)).astype(np.float32)
    Wf = (conv1_w * inv[:, None]).astype(np.float32)          # [192, 64]
    bf = (conv1_b * inv + (bn_beta - bn_mean * inv)).astype(np.float32)
    scale = np.float32(1.0 / np.sqrt(np.float32(C)))
    WQ = np.zeros((65, 64), np.float32)
    WQ[0:64] = Wf[0:64].T * scale
    WQ[64] = bf[0:64] * scale
    WK = np.zeros((65, 64), np.float32)
    WK[0:64] = Wf[64:128].T
    WK[64] = bf[64:128]
    WV = np.zeros((65, 64), np.float32)
    WV[0:64] = Wf[128:192].T
    WV[64] = bf[128:192]
    FCE = np.zeros((128, 64), np.float32)
    FCO = np.zeros((128, 64), np.float32)
    FCE[0:64] = fc_w.T.astype(np.float32)
    FCO[64:128] = fc_w.T.astype(np.float32)
    FCB = fc_b.reshape(64, 1).astype(np.float32)
    ones9 = np.zeros((128, 9 * 18), np.float32)
    for n in range(9):
        for par in range(2):
            ones9[64 * par:64 * par + 64, 18 * n + 2 * n + par] = 1.0
    zsel = np.zeros((18, 2), np.float32)
    for p in range(18):
        zsel[p, p % 2] = 1.0
    zrep = np.zeros((2, 18), np.float32)
    for p in range(18):
        zrep[p % 2, p] = 1.0
    bsel = np.zeros((18, 9 * 128), np.float32)
    for n in range(9):
        for j in range(128):
            bsel[2 * n + j // 64, 128 * n + j] = 1.0
    return dict(WQ=WQ, WK=WK, WV=WV, FCE=FCE, FCO=FCO, FCB=FCB, ONES9=ones9,
                ZSEL=zsel, ZREP=zrep, BSEL=bsel)


def build(nc: bass.Bass, qk_dt=mybir.dt.float16):
    ei = lambda n, s: nc.dram_tensor(n, s, F32, kind="ExternalInput")
    e_map = ei("e_map", [C, H, W])
    f_map = ei("f_map", [C, H, W])
    WQ, WK, WV = ei("WQ", [65, 64]), ei("WK", [65, 64]), ei("WV", [65, 64])
    FCE, FCO, FCB = ei("FCE", [128, 64]), ei("FCO", [128, 64]), ei("FCB", [64, 1])
    ONES9 = nc.dram_tensor("ONES9", [128, 9 * 18], qk_dt, kind="ExternalInput")
    ZSEL, ZREP = ei("ZSEL", [18, 2]), ei("ZREP", [2, 18])
    BSEL = ei("BSEL", [18, 9 * 128])
    y = nc.dram_tensor("y", [C, H, W], F32, kind="ExternalOutput")

    with TileContext(nc) as tc:
        with tc.tile_pool(name="persist", bufs=1) as P:
            q2 = P.tile([128, QF], qk_dt, tag="q2")
            k2 = P.tile([128, KF], qk_dt, tag="k2")
            v2 = P.tile([128, KF], F32, tag="v2")
            wq = P.tile([65, 64], F32, tag="wq")
            wk = P.tile([65, 64], F32, tag="wk")
            wv = P.tile([65, 64], F32, tag="wv")
            fce = P.tile([128, 64], F32, tag="fce")
            fco = P.tile([128, 64], F32, tag="fco")
            fcb = P.tile([64, 1], F32, tag="fcb")
            ones_t = P.tile([128, 9 * 18], qk_dt, tag="ones")
            zsel_t = P.tile([18, 2], F32, tag="zsel")
            zrep_t = P.tile([2, 18], F32, tag="zrep")
            bsel_t = P.tile([18, 9 * 128], F32, tag="bsel")

            for t, d in ((wq, WQ), (wk, WK), (wv, WV), (fce, FCE), (fco, FCO),
                         (fcb, FCB), (ones_t, ONES9), (zsel_t, ZSEL),
                         (zrep_t, ZREP), (bsel_t, BSEL)):
                nc.sync.dma_start(t[:, :], d[:, :])
            nc.gpsimd.memset(k2[:, :], 0.0)
            nc.gpsimd.memset(v2[:, :], 0.0)

            q2r = q2[:, :].rearrange("p (r w) -> p r w", w=W)
            k2r = k2[:, :].rearrange("p (r w) -> p r w", w=KW)
            v2r = v2[:, :].rearrange("p (r w) -> p r w", w=KW)

            # ---------------- conv phase (parity-split) ----------------
            with tc.tile_pool(name="est", bufs=1) as E:
                est = E.tile([65, HW], F32, tag="est")
                nc.sync.dma_start(est[0:64, :], e_map[:, :, :])
                nc.gpsimd.memset(est[64:65, :], 1.0)
                estr = est[:, :].rearrange("p (h w) -> p h w", w=W)
                with tc.tile_pool(name="cps", bufs=2, space="PSUM") as CP:
                    nrow = H // CCH              # 8 rows per chunk
                    nr2 = nrow // 2              # 4 row-pairs
                    cfa = nr2 * W                # 512 packed cols
                    for ch in range(CCH):
                        h0 = ch * nrow
                        rp0 = h0 // 2
                        pq = CP.tile([128, cfa], F32, tag="pq")
                        pk = CP.tile([128, cfa], F32, tag="pk")
                        pv = CP.tile([128, cfa], F32, tag="pv")
                        for par in (0, 1):
                            rhs = estr[:, h0 + par:h0 + nrow:2, :]
                            ps = slice(64 * par, 64 * par + 64)
                            nc.tensor.matmul(pq[ps, :], wq[:, :], rhs,
                                             start=True, stop=True)
                            nc.tensor.matmul(pk[ps, :], wk[:, :], rhs,
                                             start=True, stop=True)
                            nc.tensor.matmul(pv[ps, :], wv[:, :], rhs,
                                             start=True, stop=True)
                        nc.scalar.activation(
                            q2r[:, rp0:rp0 + nr2, :],
                            pq[:, :].rearrange("p (r w) -> p r w", w=W),
                            mybir.ActivationFunctionType.Relu)
                        nc.scalar.activation(
                            k2r[:, rp0 + 1:rp0 + 1 + nr2, 2:2 + W],
                            pk[:, :].rearrange("p (r w) -> p r w", w=W),
                            mybir.ActivationFunctionType.Relu)
                        nc.vector.tensor_scalar_max(
                            v2r[:, rp0 + 1:rp0 + 1 + nr2, 2:2 + W],
                            pv[:, :].rearrange("p (r w) -> p r w", w=W), 0.0)

            # ---------------- attention + residual + fc ----------------
            with tc.tile_pool(name="xp", bufs=1) as X:
                xp = X.tile([128, QF], F32, tag="xp")
                for par in (0, 1):
                    nc.sync.dma_start(
                        xp[64 * par:64 * par + 64, :].rearrange(
                            "p (r w) -> p r w", w=W),
                        f_map[:, par:H:2, :])

                crp = RP // ACH            # 8 row-pairs per chunk
                fa = crp * W               # 1024 packed cols
                with tc.tile_pool(name="aps", bufs=2, space="PSUM") as APS, \
                     tc.tile_pool(name="bps", bufs=3, space="PSUM") as BPS, \
                     tc.tile_pool(name="fps", bufs=2, space="PSUM") as FPS, \
                     tc.tile_pool(name="asb", bufs=3) as ASB, \
                     tc.tile_pool(name="psb", bufs=4) as PSB, \
                     tc.tile_pool(name="osb", bufs=2) as OSB:
                    for ch in range(ACH):
                        rp0 = ch * crp
                        # scores: 9 shifted q*k products -> ones-matmul
                        # partition-reduce, psum-accumulated into [18, fa]
                        s_ps = APS.tile([18, fa], F32, tag="A")
                        for n, (dy, dx) in enumerate(OFFS):
                            s = dy // 2
                            prod = PSB.tile([128, fa], qk_dt, tag="prod")
                            nc.vector.tensor_tensor(
                                prod[:, :].rearrange("p (r w) -> p r w", w=W),
                                q2r[:, rp0:rp0 + crp, :],
                                k2r[:, rp0 + 1 + s:rp0 + 1 + s + crp,
                                    2 + dx:2 + dx + W],
                                MULT)
                            for b in range(fa // 512):
                                nc.tensor.matmul(
                                    s_ps[:, b * 512:(b + 1) * 512],
                                    ones_t[:, 18 * n:18 * n + 18],
                                    prod[:, b * 512:(b + 1) * 512],
                                    start=(n == 0), stop=(n == 8))
                        # softmax over the 9 neighbors (scores are bounded
                        # small; exp without max-subtraction is safe in f32)
                        e_sb = ASB.tile([18, fa], F32, tag="e")
                        nc.scalar.activation(e_sb[:, :], s_ps[:, :],
                                             mybir.ActivationFunctionType.Exp)
                        z_ps = BPS.tile([2, fa], F32, tag="B")
                        for b in range(fa // 512):
                            nc.tensor.matmul(z_ps[:, b * 512:(b + 1) * 512],
                                             zsel_t[:, :],
                                             e_sb[:, b * 512:(b + 1) * 512],
                                             start=True, stop=True)
                        zr_sb = ASB.tile([2, fa], F32, tag="zr")
                        nc.vector.reciprocal(zr_sb[:, :], z_ps[:, :])
                        zrep_ps = APS.tile([18, fa], F32, tag="A")
                        for b in range(fa // 512):
                            nc.tensor.matmul(zrep_ps[:, b * 512:(b + 1) * 512],
                                             zrep_t[:, :],
                                             zr_sb[:, b * 512:(b + 1) * 512],
                                             start=True, stop=True)
                        a_sb = ASB.tile([18, fa], F32, tag="a")
                        nc.vector.tensor_tensor(a_sb[:, :], e_sb[:, :],
                                                zrep_ps[:, :], MULT)
                        # AV: per-n broadcast matmul + product + accumulate
                        acc = PSB.tile([128, fa], F32, tag="acc")
                        accB = PSB.tile([128, fa], F32, tag="accB")
                        for n, (dy, dx) in enumerate(OFFS):
                            s = dy // 2
                            abc = BPS.tile([128, fa], F32, tag="B")
                            for b in range(fa // 512):
                                nc.tensor.matmul(
                                    abc[:, b * 512:(b + 1) * 512],
                                    bsel_t[:, 128 * n:128 * n + 128],
                                    a_sb[:, b * 512:(b + 1) * 512],
                                    start=True, stop=True)
                            vsh = v2r[:, rp0 + 1 + s:rp0 + 1 + s + crp,
                                      2 + dx:2 + dx + W]
                            dst = (acc if n == 0 else
                                   accB if n == 1 else
                                   PSB.tile([128, fa], F32, tag="prod"))
                            nc.vector.tensor_tensor(
                                dst[:, :].rearrange("p (r w) -> p r w", w=W),
                                abc[:, :].rearrange("p (r w) -> p r w", w=W),
                                vsh, MULT)
                            if n >= 2:
                                if n % 2 == 0:
                                    nc.vector.tensor_tensor(
                                        acc[:, :], acc[:, :], dst[:, :], ADD)
                                else:
                                    nc.gpsimd.tensor_tensor(
                                        accB[:, :], accB[:, :], dst[:, :], ADD)
                        nc.gpsimd.tensor_tensor(acc[:, :], acc[:, :],
                                                accB[:, :], ADD)
                        # residual (aligned packed add)
                        nc.gpsimd.tensor_tensor(
                            xp[:, rp0 * W:(rp0 + crp) * W],
                            xp[:, rp0 * W:(rp0 + crp) * W], acc[:, :], ADD)
                        # fc (parity-split), bias at evac, strided DMA out
                        for par, fcw in ((0, fce), (1, fco)):
                            fc_ps = FPS.tile([64, fa], F32, tag="fc")
                            for b in range(fa // 512):
                                nc.tensor.matmul(
                                    fc_ps[:, b * 512:(b + 1) * 512], fcw[:, :],
                                    xp[:, rp0 * W + b * 512:
                                       rp0 * W + (b + 1) * 512],
                                    start=True, stop=True)
                            ob = OSB.tile([64, fa], F32, tag="ob")
                            nc.scalar.activation(
                                ob[:, :], fc_ps[:, :],
                                mybir.ActivationFunctionType.Identity,
                                bias=fcb[:, 0:1])
                            nc.sync.dma_start(
                                y[:, 2 * rp0 + par:2 * (rp0 + crp):2, :],
                                ob[:, :].rearrange("p (r w) -> p r w", w=W))
    return nc


_build_cache = {}


def _get_nc():
    if "nc" not in _build_cache:
        nc = bass.Bass()
        build(nc)
        _build_cache["nc"] = nc
    return _build_cache["nc"]


def run_spmd(in_maps, **kw):
    """Run the prebuilt program on cores 0..len(in_maps)-1."""
    nc = _get_nc()
    return run_bass_kernel_spmd(nc, in_maps, core_ids=list(range(len(in_maps))),
                                **kw)


def make_in_maps(f_map, e_map, conv1_w, conv1_b, bn_gamma, bn_beta, bn_mean,
                 bn_var, fc_w, fc_b):
    consts = _host_consts(np.asarray(conv1_w), np.asarray(conv1_b),
                          np.asarray(bn_gamma), np.asarray(bn_beta),
                          np.asarray(bn_mean), np.asarray(bn_var),
                          np.asarray(fc_w), np.asarray(fc_b))
    f_map = np.ascontiguousarray(np.asarray(f_map, dtype=np.float32))
    e_map = np.ascontiguousarray(np.asarray(e_map, dtype=np.float32))
    consts["ONES9"] = consts["ONES9"].astype(
        np.float16 if _build_cache.get("qk16", True) else np.float32)
    return [dict(e_map=e_map[b], f_map=f_map[b], **consts) for b in range(B)]


def kernel(f_map, e_map, conv1_w, conv1_b, bn_gamma, bn_beta, bn_mean, bn_var,
           fc_w, fc_b):
    in_maps = make_in_maps(f_map, e_map, conv1_w, conv1_b, bn_gamma, bn_beta,
                           bn_mean, bn_var, fc_w, fc_b)
    res = run_spmd(in_maps)
    out = np.stack([res.results[b]["y"] for b in range(B)]).astype(np.float32)
    return out


# revision 7
# speedup vs baseline: 6537.1334x; 1.0176x over previous
"""Trainium2 Bass kernel for nn_DEAM_79044578116356 (dilated 9-neighbor local
attention block: conv1x1+BN+ReLU -> qkv -> 3x3 dil-2 neighborhood softmax
attention -> residual -> 1x1 fc).

Contract: kernel(**inputs) takes the FULL unsharded inputs (B=8) and returns
the FULL [8, 64, 128, 128] float32 output. Internally shards data-parallel
over batch across the 8 NeuronCores (weights replicated), one image per core.

Self-contained: builds the Bass program, folds BN/bias/scale into host-side
constant tensors, runs via concourse.bass_utils.run_bass_kernel_spmd.

Device layout (per core): partition = c + 64*(h%2), free = rp*W + w with
rp = h//2. dy shifts in {-2,0,2} preserve row parity, so every dilated
(dy,dx) shift of k/v is a pure free-dim offset into a zero-padded
[66 rp x 132 w] plane. All compute-engine ops are partition-aligned; conv
and fc are parity-split with explicit psum base partitions / zero-padded
weights.
"""
import os

import numpy as np

os.environ.setdefault("JAX_COMPILATION_CACHE_DIR", "/tmp/jax_neff_cache")

import concourse.bass as bass
import concourse.mybir as mybir
from concourse.bass_utils import run_bass_kernel_spmd
from concourse.tile import TileContext

# ---------------------------------------------------------------------------
# Workaround for this walrus build's 1-sync-wait-per-instruction limit
# ("Too many sync wait commands" from setupSyncWait for CTRL/S3_LW/...).
# Extra sem waits are hoisted onto same-engine InstNoOp instructions placed
# immediately before the owner (engines run in program order, so an earlier
# same-engine wait is equivalent).
# ---------------------------------------------------------------------------
import concourse.tile as _tile_mod
from concourse.vector_clock import ScopedClock as _ScopedClock

_MAX_WAITS = 1


def _split_inst_waits(nc, inst, out_list):
    si = inst.sync_info
    if si is None or not si.on_wait or len(si.on_wait) <= _MAX_WAITS:
        out_list.append(inst)
        return
    waits = list(si.on_wait)
    keep, extra = waits[:_MAX_WAITS], waits[_MAX_WAITS:]
    si.on_wait.clear()
    si.on_wait.extend(keep)
    for i in range(0, len(extra), _MAX_WAITS):
        chunk = extra[i:i + _MAX_WAITS]
        nop = mybir.InstNoOp(
            name=nc.get_next_instruction_name(),
            engine=inst.engine,
            ins=[],
            outs=[],
            sync_info=mybir.SyncInfo(on_wait=list(chunk), on_update=[]),
            bass_nofuse=True,
        )
        nc.register_instruction(nop, overwrite=True)
        out_list.append(nop)
    out_list.append(inst)


if not getattr(_tile_mod.TileContext, "_deam_wait_patch", False):
    _orig_lower = _tile_mod.TileContext._lower_ordered_insts

    def _patched_lower(self, ordered):
        nc = self.nc
        for _bb, insts in ordered.items():
            new_list = []
            for inst in insts:
                _split_inst_waits(nc, inst, new_list)
            insts[:] = new_list
        return _orig_lower(self, ordered)

    def _patched_drain_and_barrier(self, tick_clock, wait_clock):
        nc = self.nc
        drain_inst = nc.sync.drain()
        wait_clock.add_sem_waits(
            drain_inst.ins, _ScopedClock({None: tick_clock.global_clock})
        )
        inst = drain_inst.ins
        si = inst.sync_info
        if si is not None and si.on_wait and len(si.on_wait) > _MAX_WAITS:
            waits = list(si.on_wait)
            si.on_wait.clear()
            si.on_wait.extend(waits[:_MAX_WAITS])
            rest = waits[_MAX_WAITS:]
            while rest:
                chunk, rest = rest[:_MAX_WAITS], rest[_MAX_WAITS:]
                nop = nc.sync.nop(nofuse=True, hint="drain_wait_split")
                nsi = nop.ins.sync_info
                if nsi is None:
                    nop.ins.sync_info = mybir.SyncInfo(on_wait=list(chunk),
                                                       on_update=[])
                else:
                    nsi.on_wait.extend(chunk)
        nc.all_engine_barrier()
        assert self.sems is not None
        popped = nc._tile_sem_poison_stack.pop()
        assert popped is self._sem_poison
        nc.clear_and_free_semaphores(list(self.sems.allocated().values()))
        nc.all_engine_barrier()

    _tile_mod.TileContext._lower_ordered_insts = _patched_lower
    _tile_mod.TileContext._drain_and_barrier = _patched_drain_and_barrier
    _tile_mod.TileContext._deam_wait_patch = True

# ---------------------------------------------------------------------------
# Problem constants (hardcoded per the harness contract)
# ---------------------------------------------------------------------------
F32 = mybir.dt.float32
B = 8
C, H, W = 64, 128, 128
HW = H * W
RP = H // 2            # 64 row-pairs
KW = W + 4             # 132 (w padded by 2 each side)
KR = RP + 2            # 66  (rp padded by 1 each side)
KF = KR * KW
QF = RP * W
BN_EPS = 1e-5
OFFS = [(dy, dx) for dy in (-2, 0, 2) for dx in (-2, 0, 2)]
ACH = 16               # attention chunks (4 rp each)
CCH = 16               # conv chunks (8 image rows each)
MULT = mybir.AluOpType.mult
ADD = mybir.AluOpType.add


def _host_consts(conv1_w, conv1_b, bn_gamma, bn_beta, bn_mean, bn_var,
                 fc_w, fc_b):
    """Fold BN into the conv, 1/sqrt(C) into the q chunk; build the constant
    selection matrices for the on-device partition-reduce/broadcast matmuls."""
    inv = (bn_gamma / np.sqrt(bn_var + BN_EPS)).astype(np.float32)
    Wf = (conv1_w * inv[:, None]).astype(np.float32)          # [192, 64]
    bf = (conv1_b * inv + (bn_beta - bn_mean * inv)).astype(np.float32)
    scale = np.float32(1.0 / np.sqrt(np.float32(C)))
    WQ = np.zeros((65, 64), np.float32)
    WQ[0:64] = Wf[0:64].T * scale
    WQ[64] = bf[0:64] * scale
    WK = np.zeros((65, 64), np.float32)
    WK[0:64] = Wf[64:128].T
    WK[64] = bf[64:128]
    WV = np.zeros((65, 64), np.float32)
    WV[0:64] = Wf[128:192].T
    WV[64] = bf[128:192]
    FCE = np.zeros((128, 64), np.float32)
    FCO = np.zeros((128, 64), np.float32)
    FCE[0:64] = fc_w.T.astype(np.float32)
    FCO[64:128] = fc_w.T.astype(np.float32)
    FCB = fc_b.reshape(64, 1).astype(np.float32)
    ones9 = np.zeros((128, 9 * 18), np.float32)
    for n in range(9):
        for par in range(2):
            ones9[64 * par:64 * par + 64, 18 * n + 2 * n + par] = 1.0
    zsel = np.zeros((18, 2), np.float32)
    for p in range(18):
        zsel[p, p % 2] = 1.0
    zrep = np.zeros((2, 18), np.float32)
    for p in range(18):
        zrep[p % 2, p] = 1.0
    bsel = np.zeros((18, 9 * 128), np.float32)
    for n in range(9):
        for j in range(128):
            bsel[2 * n + j // 64, 128 * n + j] = 1.0
    return dict(WQ=WQ, WK=WK, WV=WV, FCE=FCE, FCO=FCO, FCB=FCB, ONES9=ones9,
                ZSEL=zsel, ZREP=zrep, BSEL=bsel)


def build(nc: bass.Bass, qk_dt=mybir.dt.float16):
    ei = lambda n, s: nc.dram_tensor(n, s, F32, kind="ExternalInput")
    e_map = ei("e_map", [C, H, W])
    f_map = ei("f_map", [C, H, W])
    WQ, WK, WV = ei("WQ", [65, 64]), ei("WK", [65, 64]), ei("WV", [65, 64])
    FCE, FCO, FCB = ei("FCE", [128, 64]), ei("FCO", [128, 64]), ei("FCB", [64, 1])
    ONES9 = nc.dram_tensor("ONES9", [128, 9 * 18], qk_dt, kind="ExternalInput")
    ZSEL, ZREP = ei("ZSEL", [18, 2]), ei("ZREP", [2, 18])
    BSEL = ei("BSEL", [18, 9 * 128])
    y = nc.dram_tensor("y", [C, H, W], F32, kind="ExternalOutput")

    with TileContext(nc) as tc:
        with tc.tile_pool(name="persist", bufs=1) as P:
            q2 = P.tile([128, QF], qk_dt, tag="q2")
            k2 = P.tile([128, KF], qk_dt, tag="k2")
            v2 = P.tile([128, KF], qk_dt, tag="v2")
            wq = P.tile([65, 64], F32, tag="wq")
            wk = P.tile([65, 64], F32, tag="wk")
            wv = P.tile([65, 64], F32, tag="wv")
            fce = P.tile([128, 64], F32, tag="fce")
            fco = P.tile([128, 64], F32, tag="fco")
            fcb = P.tile([64, 1], F32, tag="fcb")
            ones_t = P.tile([128, 9 * 18], qk_dt, tag="ones")
            zsel_t = P.tile([18, 2], F32, tag="zsel")
            zrep_t = P.tile([2, 18], F32, tag="zrep")
            bsel_t = P.tile([18, 9 * 128], F32, tag="bsel")

            for t, d in ((wq, WQ), (wk, WK), (wv, WV), (fce, FCE), (fco, FCO),
                         (fcb, FCB), (ones_t, ONES9), (zsel_t, ZSEL),
                         (zrep_t, ZREP), (bsel_t, BSEL)):
                nc.sync.dma_start(t[:, :], d[:, :])
            nc.gpsimd.memset(k2[:, :], 0.0)
            nc.gpsimd.memset(v2[:, :], 0.0)

            q2r = q2[:, :].rearrange("p (r w) -> p r w", w=W)
            k2r = k2[:, :].rearrange("p (r w) -> p r w", w=KW)
            v2r = v2[:, :].rearrange("p (r w) -> p r w", w=KW)

            # ---------------- conv phase (parity-split) ----------------
            with tc.tile_pool(name="est", bufs=1) as E:
                est = E.tile([65, HW], F32, tag="est")
                nc.sync.dma_start(est[0:64, :], e_map[:, :, :])
                nc.gpsimd.memset(est[64:65, :], 1.0)
                estr = est[:, :].rearrange("p (h w) -> p h w", w=W)
                with tc.tile_pool(name="cps", bufs=2, space="PSUM") as CP:
                    nrow = H // CCH              # 8 rows per chunk
                    nr2 = nrow // 2              # 4 row-pairs
                    cfa = nr2 * W                # 512 packed cols
                    for ch in range(CCH):
                        h0 = ch * nrow
                        rp0 = h0 // 2
                        pq = CP.tile([128, cfa], F32, tag="pq")
                        pk = CP.tile([128, cfa], F32, tag="pk")
                        pv = CP.tile([128, cfa], F32, tag="pv")
                        for par in (0, 1):
                            rhs = estr[:, h0 + par:h0 + nrow:2, :]
                            ps = slice(64 * par, 64 * par + 64)
                            nc.tensor.matmul(pq[ps, :], wq[:, :], rhs,
                                             start=True, stop=True)
                            nc.tensor.matmul(pk[ps, :], wk[:, :], rhs,
                                             start=True, stop=True)
                            nc.tensor.matmul(pv[ps, :], wv[:, :], rhs,
                                             start=True, stop=True)
                        nc.scalar.activation(
                            q2r[:, rp0:rp0 + nr2, :],
                            pq[:, :].rearrange("p (r w) -> p r w", w=W),
                            mybir.ActivationFunctionType.Relu)
                        nc.scalar.activation(
                            k2r[:, rp0 + 1:rp0 + 1 + nr2, 2:2 + W],
                            pk[:, :].rearrange("p (r w) -> p r w", w=W),
                            mybir.ActivationFunctionType.Relu)
                        nc.vector.tensor_scalar_max(
                            v2r[:, rp0 + 1:rp0 + 1 + nr2, 2:2 + W],
                            pv[:, :].rearrange("p (r w) -> p r w", w=W), 0.0)

            # ---------------- attention + residual + fc ----------------
            with tc.tile_pool(name="xp", bufs=1) as X:
                xp = X.tile([128, QF], F32, tag="xp")
                for par in (0, 1):
                    nc.sync.dma_start(
                        xp[64 * par:64 * par + 64, :].rearrange(
                            "p (r w) -> p r w", w=W),
                        f_map[:, par:H:2, :])

                crp = RP // ACH            # 8 row-pairs per chunk
                fa = crp * W               # 1024 packed cols
                with tc.tile_pool(name="aps", bufs=2, space="PSUM") as APS, \
                     tc.tile_pool(name="bps", bufs=3, space="PSUM") as BPS, \
                     tc.tile_pool(name="fps", bufs=2, space="PSUM") as FPS, \
                     tc.tile_pool(name="asb", bufs=3) as ASB, \
                     tc.tile_pool(name="psb", bufs=4) as PSB, \
                     tc.tile_pool(name="osb", bufs=2) as OSB:
                    for ch in range(ACH):
                        rp0 = ch * crp
                        # scores: 9 shifted q*k products -> ones-matmul
                        # partition-reduce, psum-accumulated into [18, fa]
                        s_ps = APS.tile([18, fa], F32, tag="A")
                        for n, (dy, dx) in enumerate(OFFS):
                            s = dy // 2
                            prod = PSB.tile([128, fa], qk_dt, tag="prod")
                            nc.vector.tensor_tensor(
                                prod[:, :].rearrange("p (r w) -> p r w", w=W),
                                q2r[:, rp0:rp0 + crp, :],
                                k2r[:, rp0 + 1 + s:rp0 + 1 + s + crp,
                                    2 + dx:2 + dx + W],
                                MULT)
                            for b in range(fa // 512):
                                nc.tensor.matmul(
                                    s_ps[:, b * 512:(b + 1) * 512],
                                    ones_t[:, 18 * n:18 * n + 18],
                                    prod[:, b * 512:(b + 1) * 512],
                                    start=(n == 0), stop=(n == 8))
                        # softmax over the 9 neighbors (scores are bounded
                        # small; exp without max-subtraction is safe in f32)
                        e_sb = ASB.tile([18, fa], F32, tag="e")
                        nc.scalar.activation(e_sb[:, :], s_ps[:, :],
                                             mybir.ActivationFunctionType.Exp)
                        z_ps = BPS.tile([2, fa], F32, tag="B")
                        for b in range(fa // 512):
                            nc.tensor.matmul(z_ps[:, b * 512:(b + 1) * 512],
                                             zsel_t[:, :],
                                             e_sb[:, b * 512:(b + 1) * 512],
                                             start=True, stop=True)
                        zr_sb = ASB.tile([2, fa], F32, tag="zr")
                        nc.vector.reciprocal(zr_sb[:, :], z_ps[:, :])
                        zrep_ps = APS.tile([18, fa], F32, tag="A")
                        for b in range(fa // 512):
                            nc.tensor.matmul(zrep_ps[:, b * 512:(b + 1) * 512],
                                             zrep_t[:, :],
                                             zr_sb[:, b * 512:(b + 1) * 512],
                                             start=True, stop=True)
                        a_sb = ASB.tile([18, fa], F32, tag="a")
                        nc.vector.tensor_tensor(a_sb[:, :], e_sb[:, :],
                                                zrep_ps[:, :], MULT)
                        # AV: per-n broadcast matmul + product + accumulate
                        acc = PSB.tile([128, fa], qk_dt, tag="acc")
                        accB = PSB.tile([128, fa], qk_dt, tag="accB")
                        accF = PSB.tile([128, fa], F32, tag="accF")
                        for n, (dy, dx) in enumerate(OFFS):
                            s = dy // 2
                            abc = BPS.tile([128, fa], F32, tag="B")
                            for b in range(fa // 512):
                                nc.tensor.matmul(
                                    abc[:, b * 512:(b + 1) * 512],
                                    bsel_t[:, 128 * n:128 * n + 128],
                                    a_sb[:, b * 512:(b + 1) * 512],
                                    start=True, stop=True)
                            abc16 = ASB.tile([128, fa], qk_dt, tag="abc16")
                            nc.scalar.activation(
                                abc16[:, :], abc[:, :],
                                mybir.ActivationFunctionType.Copy)
                            vsh = v2r[:, rp0 + 1 + s:rp0 + 1 + s + crp,
                                      2 + dx:2 + dx + W]
                            dst = (acc if n == 0 else
                                   accB if n == 1 else
                                   PSB.tile([128, fa], qk_dt, tag="prod"))
                            nc.vector.tensor_tensor(
                                dst[:, :].rearrange("p (r w) -> p r w", w=W),
                                abc16[:, :].rearrange("p (r w) -> p r w", w=W),
                                vsh, MULT)
                            if n >= 2:
                                if n % 2 == 0:
                                    nc.vector.tensor_tensor(
                                        acc[:, :], acc[:, :], dst[:, :], ADD)
                                else:
                                    nc.gpsimd.tensor_tensor(
                                        accB[:, :], accB[:, :], dst[:, :], ADD)
                        nc.vector.tensor_tensor(accF[:, :], acc[:, :],
                                                accB[:, :], ADD)
                        # residual (aligned packed add)
                        nc.gpsimd.tensor_tensor(
                            xp[:, rp0 * W:(rp0 + crp) * W],
                            xp[:, rp0 * W:(rp0 + crp) * W], accF[:, :], ADD)
                        # fc (parity-split), bias at evac, strided DMA out
                        for par, fcw in ((0, fce), (1, fco)):
                            fc_ps = FPS.tile([64, fa], F32, tag="fc")
                            for b in range(fa // 512):
                                nc.tensor.matmul(
                                    fc_ps[:, b * 512:(b + 1) * 512], fcw[:, :],
                                    xp[:, rp0 * W + b * 512:
                                       rp0 * W + (b + 1) * 512],
                                    start=True, stop=True)
                            ob = OSB.tile([64, fa], F32, tag="ob")
                            nc.scalar.activation(
                                ob[:, :], fc_ps[:, :],
                                mybir.ActivationFunctionType.Identity,
                                bias=fcb[:, 0:1])
                            nc.sync.dma_start(
                                y[:, 2 * rp0 + par:2 * (rp0 + crp):2, :],
                                ob[:, :].rearrange("p (r w) -> p r w", w=W))
    return nc


_build_cache = {}


def _get_nc():
    if "nc" not in _build_cache:
        nc = bass.Bass()
        build(nc)
        _build_cache["nc"] = nc
    return _build_cache["nc"]


def run_spmd(in_maps, **kw):
    """Run the prebuilt program on cores 0..len(in_maps)-1."""
    nc = _get_nc()
    return run_bass_kernel_spmd(nc, in_maps, core_ids=list(range(len(in_maps))),
                                **kw)


def make_in_maps(f_map, e_map, conv1_w, conv1_b, bn_gamma, bn_beta, bn_mean,
                 bn_var, fc_w, fc_b):
    consts = _host_consts(np.asarray(conv1_w), np.asarray(conv1_b),
                          np.asarray(bn_gamma), np.asarray(bn_beta),
                          np.asarray(bn_mean), np.asarray(bn_var),
                          np.asarray(fc_w), np.asarray(fc_b))
    f_map = np.ascontiguousarray(np.asarray(f_map, dtype=np.float32))
    e_map = np.ascontiguousarray(np.asarray(e_map, dtype=np.float32))
    consts["ONES9"] = consts["ONES9"].astype(
        np.float16 if _build_cache.get("qk16", True) else np.float32)
    return [dict(e_map=e_map[b], f_map=f_map[b], **consts) for b in range(B)]


def kernel(f_map, e_map, conv1_w, conv1_b, bn_gamma, bn_beta, bn_mean, bn_var,
           fc_w, fc_b):
    in_maps = make_in_maps(f_map, e_map, conv1_w, conv1_b, bn_gamma, bn_beta,
                           bn_mean, bn_var, fc_w, fc_b)
    res = run_spmd(in_maps)
    out = np.stack([res.results[b]["y"] for b in range(B)]).astype(np.float32)
    return out


# revision 8
# speedup vs baseline: 6545.1453x; 1.0012x over previous
"""Trainium2 Bass kernel for nn_DEAM_79044578116356 (dilated 9-neighbor local
attention block: conv1x1+BN+ReLU -> qkv -> 3x3 dil-2 neighborhood softmax
attention -> residual -> 1x1 fc).

Contract: kernel(**inputs) takes the FULL unsharded inputs (B=8) and returns
the FULL [8, 64, 128, 128] float32 output. Internally shards data-parallel
over batch across the 8 NeuronCores (weights replicated), one image per core.

Self-contained: builds the Bass program, folds BN/bias/scale into host-side
constant tensors, runs via concourse.bass_utils.run_bass_kernel_spmd.

Device layout (per core): partition = c + 64*(h%2), free = rp*W + w with
rp = h//2. dy shifts in {-2,0,2} preserve row parity, so every dilated
(dy,dx) shift of k/v is a pure free-dim offset into a zero-padded
[66 rp x 132 w] plane. All compute-engine ops are partition-aligned; conv
and fc are parity-split with explicit psum base partitions / zero-padded
weights.
"""
import os

import numpy as np

os.environ.setdefault("JAX_COMPILATION_CACHE_DIR", "/tmp/jax_neff_cache")

import concourse.bass as bass
import concourse.mybir as mybir
from concourse.bass_utils import run_bass_kernel_spmd
from concourse.tile import TileContext

# ---------------------------------------------------------------------------
# Workaround for this walrus build's 1-sync-wait-per-instruction limit
# ("Too many sync wait commands" from setupSyncWait for CTRL/S3_LW/...).
# Extra sem waits are hoisted onto same-engine InstNoOp instructions placed
# immediately before the owner (engines run in program order, so an earlier
# same-engine wait is equivalent).
# ---------------------------------------------------------------------------
import concourse.tile as _tile_mod
from concourse.vector_clock import ScopedClock as _ScopedClock

_MAX_WAITS = 1


def _split_inst_waits(nc, inst, out_list):
    si = inst.sync_info
    if si is None or not si.on_wait or len(si.on_wait) <= _MAX_WAITS:
        out_list.append(inst)
        return
    waits = list(si.on_wait)
    keep, extra = waits[:_MAX_WAITS], waits[_MAX_WAITS:]
    si.on_wait.clear()
    si.on_wait.extend(keep)
    for i in range(0, len(extra), _MAX_WAITS):
        chunk = extra[i:i + _MAX_WAITS]
        nop = mybir.InstNoOp(
            name=nc.get_next_instruction_name(),
            engine=inst.engine,
            ins=[],
            outs=[],
            sync_info=mybir.SyncInfo(on_wait=list(chunk), on_update=[]),
            bass_nofuse=True,
        )
        nc.register_instruction(nop, overwrite=True)
        out_list.append(nop)
    out_list.append(inst)


if not getattr(_tile_mod.TileContext, "_deam_wait_patch", False):
    _orig_lower = _tile_mod.TileContext._lower_ordered_insts

    def _patched_lower(self, ordered):
        nc = self.nc
        for _bb, insts in ordered.items():
            new_list = []
            for inst in insts:
                _split_inst_waits(nc, inst, new_list)
            insts[:] = new_list
        return _orig_lower(self, ordered)

    def _patched_drain_and_barrier(self, tick_clock, wait_clock):
        nc = self.nc
        drain_inst = nc.sync.drain()
        wait_clock.add_sem_waits(
            drain_inst.ins, _ScopedClock({None: tick_clock.global_clock})
        )
        inst = drain_inst.ins
        si = inst.sync_info
        if si is not None and si.on_wait and len(si.on_wait) > _MAX_WAITS:
            waits = list(si.on_wait)
            si.on_wait.clear()
            si.on_wait.extend(waits[:_MAX_WAITS])
            rest = waits[_MAX_WAITS:]
            while rest:
                chunk, rest = rest[:_MAX_WAITS], rest[_MAX_WAITS:]
                nop = nc.sync.nop(nofuse=True, hint="drain_wait_split")
                nsi = nop.ins.sync_info
                if nsi is None:
                    nop.ins.sync_info = mybir.SyncInfo(on_wait=list(chunk),
                                                       on_update=[])
                else:
                    nsi.on_wait.extend(chunk)
        nc.all_engine_barrier()
        assert self.sems is not None
        popped = nc._tile_sem_poison_stack.pop()
        assert popped is self._sem_poison
        nc.clear_and_free_semaphores(list(self.sems.allocated().values()))
        nc.all_engine_barrier()

    _tile_mod.TileContext._lower_ordered_insts = _patched_lower
    _tile_mod.TileContext._drain_and_barrier = _patched_drain_and_barrier
    _tile_mod.TileContext._deam_wait_patch = True

# ---------------------------------------------------------------------------
# Problem constants (hardcoded per the harness contract)
# ---------------------------------------------------------------------------
F32 = mybir.dt.float32
B = 8
C, H, W = 64, 128, 128
HW = H * W
RP = H // 2            # 64 row-pairs
KW = W + 4             # 132 (w padded by 2 each side)
KR = RP + 2            # 66  (rp padded by 1 each side)
KF = KR * KW
QF = RP * W
BN_EPS = 1e-5
OFFS = [(dy, dx) for dy in (-2, 0, 2) for dx in (-2, 0, 2)]
ACH = 16               # attention chunks (4 rp each)
CCH = 16               # conv chunks (8 image rows each)
MULT = mybir.AluOpType.mult
ADD = mybir.AluOpType.add


def _host_consts(conv1_w, conv1_b, bn_gamma, bn_beta, bn_mean, bn_var,
                 fc_w, fc_b):
    """Fold BN into the conv, 1/sqrt(C) into the q chunk; build the constant
    selection matrices for the on-device partition-reduce/broadcast matmuls."""
    inv = (bn_gamma / np.sqrt(bn_var + BN_EPS)).astype(np.float32)
    Wf = (conv1_w * inv[:, None]).astype(np.float32)          # [192, 64]
    bf = (conv1_b * inv + (bn_beta - bn_mean * inv)).astype(np.float32)
    scale = np.float32(1.0 / np.sqrt(np.float32(C)))
    WQ = np.zeros((65, 64), np.float32)
    WQ[0:64] = Wf[0:64].T * scale
    WQ[64] = bf[0:64] * scale
    WK = np.zeros((65, 64), np.float32)
    WK[0:64] = Wf[64:128].T
    WK[64] = bf[64:128]
    WV = np.zeros((65, 64), np.float32)
    WV[0:64] = Wf[128:192].T
    WV[64] = bf[128:192]
    FCE = np.zeros((128, 64), np.float32)
    FCO = np.zeros((128, 64), np.float32)
    FCE[0:64] = fc_w.T.astype(np.float32)
    FCO[64:128] = fc_w.T.astype(np.float32)
    FCB = fc_b.reshape(64, 1).astype(np.float32)
    ones9 = np.zeros((128, 9 * 18), np.float32)
    for n in range(9):
        for par in range(2):
            ones9[64 * par:64 * par + 64, 18 * n + 2 * n + par] = 1.0
    zsel = np.zeros((18, 2), np.float32)
    for p in range(18):
        zsel[p, p % 2] = 1.0
    zrep = np.zeros((2, 18), np.float32)
    for p in range(18):
        zrep[p % 2, p] = 1.0
    bsel = np.zeros((18, 9 * 128), np.float32)
    for n in range(9):
        for j in range(128):
            bsel[2 * n + j // 64, 128 * n + j] = 1.0
    return dict(WQ=WQ, WK=WK, WV=WV, FCE=FCE, FCO=FCO, FCB=FCB, ONES9=ones9,
                ZSEL=zsel, ZREP=zrep, BSEL=bsel)


def build(nc: bass.Bass, qk_dt=mybir.dt.float16):
    ei = lambda n, s: nc.dram_tensor(n, s, F32, kind="ExternalInput")
    e_map = ei("e_map", [C, H, W])
    f_map = ei("f_map", [C, H, W])
    WQ, WK, WV = ei("WQ", [65, 64]), ei("WK", [65, 64]), ei("WV", [65, 64])
    FCE, FCO, FCB = ei("FCE", [128, 64]), ei("FCO", [128, 64]), ei("FCB", [64, 1])
    ONES9 = nc.dram_tensor("ONES9", [128, 9 * 18], qk_dt, kind="ExternalInput")
    ZSEL, ZREP = ei("ZSEL", [18, 2]), ei("ZREP", [2, 18])
    BSEL = ei("BSEL", [18, 9 * 128])
    y = nc.dram_tensor("y", [C, H, W], F32, kind="ExternalOutput")

    with TileContext(nc) as tc:
        with tc.tile_pool(name="persist", bufs=1) as P:
            q2 = P.tile([128, QF], qk_dt, tag="q2")
            k2 = P.tile([128, KF], qk_dt, tag="k2")
            v2 = P.tile([128, KF], qk_dt, tag="v2")
            wq = P.tile([65, 64], F32, tag="wq")
            wk = P.tile([65, 64], F32, tag="wk")
            wv = P.tile([65, 64], F32, tag="wv")
            fce = P.tile([128, 64], F32, tag="fce")
            fco = P.tile([128, 64], F32, tag="fco")
            fcb = P.tile([64, 1], F32, tag="fcb")
            ones_t = P.tile([128, 9 * 18], qk_dt, tag="ones")
            zsel_t = P.tile([18, 2], F32, tag="zsel")
            zrep_t = P.tile([2, 18], F32, tag="zrep")
            bsel_t = P.tile([18, 9 * 128], F32, tag="bsel")

            for t, d in ((wq, WQ), (wk, WK), (wv, WV), (fce, FCE), (fco, FCO),
                         (fcb, FCB), (ones_t, ONES9), (zsel_t, ZSEL),
                         (zrep_t, ZREP), (bsel_t, BSEL)):
                nc.sync.dma_start(t[:, :], d[:, :])
            nc.gpsimd.memset(k2[:, :], 0.0)
            nc.gpsimd.memset(v2[:, :], 0.0)

            q2r = q2[:, :].rearrange("p (r w) -> p r w", w=W)
            k2r = k2[:, :].rearrange("p (r w) -> p r w", w=KW)
            v2r = v2[:, :].rearrange("p (r w) -> p r w", w=KW)

            # ---------------- conv phase (parity-split) ----------------
            with tc.tile_pool(name="est", bufs=1) as E:
                est = E.tile([65, HW], F32, tag="est")
                nc.sync.dma_start(est[0:64, :], e_map[:, :, :])
                nc.gpsimd.memset(est[64:65, :], 1.0)
                estr = est[:, :].rearrange("p (h w) -> p h w", w=W)
                with tc.tile_pool(name="cps", bufs=2, space="PSUM") as CP:
                    nrow = H // CCH              # 8 rows per chunk
                    nr2 = nrow // 2              # 4 row-pairs
                    cfa = nr2 * W                # 512 packed cols
                    for ch in range(CCH):
                        h0 = ch * nrow
                        rp0 = h0 // 2
                        pq = CP.tile([128, cfa], F32, tag="pq")
                        pk = CP.tile([128, cfa], F32, tag="pk")
                        pv = CP.tile([128, cfa], F32, tag="pv")
                        for par in (0, 1):
                            rhs = estr[:, h0 + par:h0 + nrow:2, :]
                            ps = slice(64 * par, 64 * par + 64)
                            nc.tensor.matmul(pq[ps, :], wq[:, :], rhs,
                                             start=True, stop=True)
                            nc.tensor.matmul(pk[ps, :], wk[:, :], rhs,
                                             start=True, stop=True)
                            nc.tensor.matmul(pv[ps, :], wv[:, :], rhs,
                                             start=True, stop=True)
                        nc.scalar.activation(
                            q2r[:, rp0:rp0 + nr2, :],
                            pq[:, :].rearrange("p (r w) -> p r w", w=W),
                            mybir.ActivationFunctionType.Relu)
                        nc.scalar.activation(
                            k2r[:, rp0 + 1:rp0 + 1 + nr2, 2:2 + W],
                            pk[:, :].rearrange("p (r w) -> p r w", w=W),
                            mybir.ActivationFunctionType.Relu)
                        nc.scalar.activation(
                            v2r[:, rp0 + 1:rp0 + 1 + nr2, 2:2 + W],
                            pv[:, :].rearrange("p (r w) -> p r w", w=W),
                            mybir.ActivationFunctionType.Relu)

            # ---------------- attention + residual + fc ----------------
            with tc.tile_pool(name="xp", bufs=1) as X:
                xp = X.tile([128, QF], F32, tag="xp")
                for par in (0, 1):
                    nc.sync.dma_start(
                        xp[64 * par:64 * par + 64, :].rearrange(
                            "p (r w) -> p r w", w=W),
                        f_map[:, par:H:2, :])

                crp = RP // ACH            # 8 row-pairs per chunk
                fa = crp * W               # 1024 packed cols
                with tc.tile_pool(name="aps", bufs=2, space="PSUM") as APS, \
                     tc.tile_pool(name="bps", bufs=3, space="PSUM") as BPS, \
                     tc.tile_pool(name="fps", bufs=2, space="PSUM") as FPS, \
                     tc.tile_pool(name="asb", bufs=3) as ASB, \
                     tc.tile_pool(name="psb", bufs=4) as PSB, \
                     tc.tile_pool(name="osb", bufs=2) as OSB:
                    for ch in range(ACH):
                        rp0 = ch * crp
                        # scores: 9 shifted q*k products -> ones-matmul
                        # partition-reduce, psum-accumulated into [18, fa]
                        s_ps = APS.tile([18, fa], F32, tag="A")
                        for n, (dy, dx) in enumerate(OFFS):
                            s = dy // 2
                            prod = PSB.tile([128, fa], qk_dt, tag="prod")
                            nc.vector.tensor_tensor(
                                prod[:, :].rearrange("p (r w) -> p r w", w=W),
                                q2r[:, rp0:rp0 + crp, :],
                                k2r[:, rp0 + 1 + s:rp0 + 1 + s + crp,
                                    2 + dx:2 + dx + W],
                                MULT)
                            for b in range(fa // 512):
                                nc.tensor.matmul(
                                    s_ps[:, b * 512:(b + 1) * 512],
                                    ones_t[:, 18 * n:18 * n + 18],
                                    prod[:, b * 512:(b + 1) * 512],
                                    start=(n == 0), stop=(n == 8))
                        # softmax over the 9 neighbors (scores are bounded
                        # small; exp without max-subtraction is safe in f32)
                        e_sb = ASB.tile([18, fa], F32, tag="e")
                        nc.scalar.activation(e_sb[:, :], s_ps[:, :],
                                             mybir.ActivationFunctionType.Exp)
                        z_ps = BPS.tile([2, fa], F32, tag="B")
                        for b in range(fa // 512):
                            nc.tensor.matmul(z_ps[:, b * 512:(b + 1) * 512],
                                             zsel_t[:, :],
                                             e_sb[:, b * 512:(b + 1) * 512],
                                             start=True, stop=True)
                        zr_sb = ASB.tile([2, fa], F32, tag="zr")
                        nc.vector.reciprocal(zr_sb[:, :], z_ps[:, :])
                        zrep_ps = APS.tile([18, fa], F32, tag="A")
                        for b in range(fa // 512):
                            nc.tensor.matmul(zrep_ps[:, b * 512:(b + 1) * 512],
                                             zrep_t[:, :],
                                             zr_sb[:, b * 512:(b + 1) * 512],
                                             start=True, stop=True)
                        a_sb = ASB.tile([18, fa], F32, tag="a")
                        nc.vector.tensor_tensor(a_sb[:, :], e_sb[:, :],
                                                zrep_ps[:, :], MULT)
                        # AV: per-n broadcast matmul + product + accumulate
                        acc = PSB.tile([128, fa], qk_dt, tag="acc")
                        accB = PSB.tile([128, fa], qk_dt, tag="accB")
                        accF = PSB.tile([128, fa], F32, tag="accF")
                        for n, (dy, dx) in enumerate(OFFS):
                            s = dy // 2
                            abc = BPS.tile([128, fa], F32, tag="B")
                            for b in range(fa // 512):
                                nc.tensor.matmul(
                                    abc[:, b * 512:(b + 1) * 512],
                                    bsel_t[:, 128 * n:128 * n + 128],
                                    a_sb[:, b * 512:(b + 1) * 512],
                                    start=True, stop=True)
                            abc16 = ASB.tile([128, fa], qk_dt, tag="abc16")
                            nc.scalar.activation(
                                abc16[:, :], abc[:, :],
                                mybir.ActivationFunctionType.Copy)
                            vsh = v2r[:, rp0 + 1 + s:rp0 + 1 + s + crp,
                                      2 + dx:2 + dx + W]
                            dst = (acc if n == 0 else
                                   accB if n == 1 else
                                   PSB.tile([128, fa], qk_dt, tag="prod"))
                            nc.vector.tensor_tensor(
                                dst[:, :].rearrange("p (r w) -> p r w", w=W),
                                abc16[:, :].rearrange("p (r w) -> p r w", w=W),
                                vsh, MULT)
                            if n >= 2:
                                if n % 2 == 0:
                                    nc.vector.tensor_tensor(
                                        acc[:, :], acc[:, :], dst[:, :], ADD)
                                else:
                                    nc.gpsimd.tensor_tensor(
                                        accB[:, :], accB[:, :], dst[:, :], ADD)
                        nc.vector.tensor_tensor(accF[:, :], acc[:, :],
                                                accB[:, :], ADD)
                        # residual (aligned packed add)
                        nc.gpsimd.tensor_tensor(
                            xp[:, rp0 * W:(rp0 + crp) * W],
                            xp[:, rp0 * W:(rp0 + crp) * W], accF[:, :], ADD)
                        # fc (parity-split), bias at evac, strided DMA out
                        for par, fcw in ((0, fce), (1, fco)):
                            fc_ps = FPS.tile([64, fa], F32, tag="fc")
                            for b in range(fa // 512):
                                nc.tensor.matmul(
                                    fc_ps[:, b * 512:(b + 1) * 512], fcw[:, :],
                                    xp[:, rp0 * W + b * 512:
                                       rp0 * W + (b + 1) * 512],
                                    start=True, stop=True)
                            ob = OSB.tile([64, fa], F32, tag="ob")
                            nc.scalar.activation(
                                ob[:, :], fc_ps[:, :],
                                mybir.ActivationFunctionType.Identity,
                                bias=fcb[:, 0:1])
                            nc.sync.dma_start(
                                y[:, 2 * rp0 + par:2 * (rp0 + crp):2, :],
                                ob[:, :].rearrange("p (r w) -> p r w", w=W))
    return nc


_build_cache = {}


def _get_nc():
    if "nc" not in _build_cache:
        nc = bass.Bass()
        build(nc)
        _build_cache["nc"] = nc
    return _build_cache["nc"]


def run_spmd(in_maps, **kw):
    """Run the prebuilt program on cores 0..len(in_maps)-1."""
    nc = _get_nc()
    return run_bass_kernel_spmd(nc, in_maps, core_ids=list(range(len(in_maps))),
                                **kw)


def make_in_maps(f_map, e_map, conv1_w, conv1_b, bn_gamma, bn_beta, bn_mean,
                 bn_var, fc_w, fc_b):
    consts = _host_consts(np.asarray(conv1_w), np.asarray(conv1_b),
                          np.asarray(bn_gamma), np.asarray(bn_beta),
                          np.asarray(bn_mean), np.asarray(bn_var),
                          np.asarray(fc_w), np.asarray(fc_b))
    f_map = np.ascontiguousarray(np.asarray(f_map, dtype=np.float32))
    e_map = np.ascontiguousarray(np.asarray(e_map, dtype=np.float32))
    consts["ONES9"] = consts["ONES9"].astype(
        np.float16 if _build_cache.get("qk16", True) else np.float32)
    return [dict(e_map=e_map[b], f_map=f_map[b], **consts) for b in range(B)]


def kernel(f_map, e_map, conv1_w, conv1_b, bn_gamma, bn_beta, bn_mean, bn_var,
           fc_w, fc_b):
    in_maps = make_in_maps(f_map, e_map, conv1_w, conv1_b, bn_gamma, bn_beta,
                           bn_mean, bn_var, fc_w, fc_b)
    res = run_spmd(in_maps)
    out = np.stack([res.results[b]["y"] for b in range(B)]).astype(np.float32)
    return out
